# revision 1
# baseline (speedup 1.0000x reference)
import numpy as np

RETINA = 224.0
NUM_CLASSES = 4
B = 8
S = 2048
M = S - 1
NCORES = 8
BIG = 1.0e13
KQ12 = 4 * 6 + 2
KQ34 = 4 * 6
NCOLS = 32
CHUNK = 512
DIAG_W = 130

_CACHE = {}
TRACE_KWARGS = {}
LAST_RESULTS = None


def _split_multi_waits(nc, max_waits=1):
    import concourse.mybir as mybir
    for fn in nc.m.functions:
        for blk in fn.blocks:
            out = []
            changed = False
            for inst in blk.instructions:
                si = inst.sync_info
                ow = list(si.on_wait) if (si is not None and si.on_wait) else []
                if len(ow) > max_waits:
                    for k, w in enumerate(ow[:-max_waits]):
                        out.append(mybir.InstNoOp(
                            name=f"{inst.name}_wsplit{k}",
                            engine=inst.engine,
                            ins=[], outs=[],
                            sync_info=mybir.SyncInfo(on_wait=[w],
                                                     on_update=[]),
                        ))
                    si.on_wait = ow[-max_waits:]
                    changed = True
                out.append(inst)
            if changed:
                blk.instructions = out
    return nc


def _split2(x):
    import ml_dtypes
    bf = ml_dtypes.bfloat16
    hi = x.astype(bf).astype(np.float64)
    mid = (x - hi).astype(bf).astype(np.float64)
    return hi, mid


def _split_stack_A(A):
    h, m = _split2(A)
    import ml_dtypes
    return np.concatenate([h, m, h, m], 0).astype(ml_dtypes.bfloat16)


def _split_stack_B(Bm):
    h, m = _split2(Bm)
    import ml_dtypes
    return np.concatenate([h, h, m, m], 0).astype(ml_dtypes.bfloat16)


def _host_prep(pp, ts, pm):
    tc_cls = ts[:, :, 4].astype(np.int32)
    valid = ~pm
    nn = valid & (tc_cls != 0)

    per_core = []
    n_segs = []
    for b in range(B):
        order = np.argsort(~nn[b], kind="stable")
        pts = pp[b][order].astype(np.float64)
        n = int(nn[b].sum())
        n_seg = n - 1
        n_segs.append(n_seg)
        if n > 0:
            pts = pts - pts[:n].mean(axis=0)
        sx, sy = pts[:-1, 0], pts[:-1, 1]
        eX, eY = pts[1:, 0], pts[1:, 1]
        ex, ey = eX - sx, eY - sy
        c = ex * sy - ey * sx
        g0, g1, g2 = ex, -ey, -c
        one = np.ones(M)
        U6 = np.stack([g0 * g0, g1 * g1, g2 * g2,
                       g0 * g1, g0 * g2, g1 * g2], 0)
        V6 = np.stack([sy * eY, sx * eX, one,
                       sy * eX + sx * eY,
                       sy + eY,
                       sx + eX], 0)
        import ml_dtypes
        bfd = ml_dtypes.bfloat16
        inv = (np.arange(M) >= max(n_seg, 0)).astype(np.float64) * BIG
        A12 = np.concatenate([_split_stack_A(V6),
                              inv[None].astype(bfd), one[None].astype(bfd)], 0)
        B12 = np.concatenate([_split_stack_B(U6),
                              one[None].astype(bfd), inv[None].astype(bfd)], 0)
        per_core.append(dict(n=n, n_seg=n_seg,
                             A12=A12.astype(bfd), B12=B12.astype(bfd),
                             A34=_split_stack_A(U6), B34=_split_stack_B(V6)))
    return tc_cls, valid, nn, per_core, n_segs


def _schedule(L, Jmax):
    fulls, tails = [], []
    for ib in range(L // 128):
        i0 = 128 * ib
        j0 = i0
        while j0 < Jmax:
            N = min(CHUNK, Jmax - j0)
            (fulls if N == CHUNK else tails).append((i0, j0, N))
            j0 += N
    order = []
    fi = ti = 0
    while fi < len(fulls) or ti < len(tails):
        if fi < len(fulls):
            order.append(fulls[fi])
            fi += 1
        if ti < len(tails):
            order.append(tails[ti])
            ti += 1
    chunks = []
    pos = 0
    for (i0, j0, N) in order:
        chunks.append((i0, j0, N, pos))
        pos += N
    ops = []
    c = 0
    while c < len(chunks):
        ops.append((c, 1, chunks[c][3], chunks[c][2]))
        c += 1
    opidx = {}
    for oi, (c0, k, _, _) in enumerate(ops):
        for cc in range(c0, c0 + k):
            opidx[cc] = oi
    assert len(ops) <= NCOLS
    return chunks, ops, opidx, pos


def _build_program(L, Jmax):
    import concourse.bass as bass
    from concourse import mybir

    f32 = mybir.dt.float32
    bf16 = mybir.dt.bfloat16
    ALU = mybir.AluOpType
    ACT = mybir.ActivationFunctionType

    chunks, ops, opidx, total = _schedule(L, Jmax)
    SW = -(-total // 16) * 16

    nc = bass.Bass()
    d_feat = nc.dram_tensor("feat", [KQ12, 4 * L], bf16, kind="ExternalInput")
    NG = len(ops)
    d_out = nc.dram_tensor("partials", [128, NG], f32,
                           kind="ExternalOutput")

    NSPLIT = min(16, NG - 1)
    with (
        nc.sbuf_tensor([KQ12, 4 * L], bf16) as feat,
        nc.sbuf_tensor([KQ12, 768], bf16) as warm,
        nc.sbuf_tensor([128, SW], bf16) as sring,
        nc.sbuf_tensor([128, NCOLS], f32) as cols,
        nc.psum_tensor([128, 4096], f32) as qps,
        nc.semaphore("dma0_sem") as dma0_sem,
        nc.semaphore("dma1_sem") as dma1_sem,
        nc.semaphore("dma2_sem") as dma2_sem,
        nc.semaphore("dma3_sem") as dma3_sem,
        nc.semaphore("dmao_sem") as dmao_sem,
        nc.semaphore("dmap_sem") as dmap_sem,
        nc.semaphore("warm_sem") as warm_sem,
        nc.semaphore("strt_sem") as strt_sem,
        nc.semaphore("pe_sem") as pe_sem,
        nc.semaphore("pe12_sem") as pe12_sem,
        nc.semaphore("dve_sem") as dve_sem,
        nc.semaphore("act_sem") as act_sem,
        nc.Block() as block,
    ):
        fB12 = feat[:, 0 * L:1 * L]
        fA12 = feat[:, 1 * L:2 * L]
        fB34 = feat[0:KQ34, 2 * L:3 * L]
        fA34 = feat[0:KQ34, 3 * L:4 * L]

        @block.sync
        def _(sy):
            sy.dma_start(out=feat[:, 0:L],
                         in_=d_feat[:, 0:L]).then_inc(dma0_sem, 16)
            sy.dma_start(out=feat[:, L + 128:2 * L],
                         in_=d_feat[:, L + 128:2 * L]).then_inc(dma1_sem, 16)

        @block.gpsimd
        def _(gp):
            nc.gpsimd.memset(warm[:], 0.0).then_inc(warm_sem, 1)
            gp.dma_start(out=feat[:, 3 * L:4 * L],
                         in_=d_feat[:, 3 * L:4 * L]).then_inc(dma3_sem, 16)
            gp.wait_ge(dve_sem, NSPLIT)
            gp.dma_start(out=d_out[:, 0:NSPLIT],
                         in_=cols[:, 0:NSPLIT]).then_inc(dmap_sem, 16)
            gp.wait_ge(dmap_sem, 16)
            gp.wait_ge(dmao_sem, 16)

        @block.tensor
        def _(pe):
            def q12(c):
                i0, j0, N, _ = chunks[c]
                o12 = 512 * (c % 4)
                nc.tensor.matmul(qps[:, o12:o12 + N],
                                 fA12[:, i0:i0 + 128],
                                 fB12[:, j0:j0 + N],
                                 start=True, stop=True).then_inc(pe12_sem, 1)

            def q34(c):
                i0, j0, N, _ = chunks[c]
                o34 = 2048 + 512 * (c % 4)
                return nc.tensor.matmul(qps[:, o34:o34 + N],
                                        fA34[:, i0:i0 + 128],
                                        fB34[:, j0:j0 + N],
                                        start=True, stop=True)

            pe.wait_ge(warm_sem, 1)
            for _ in range(5):
                nc.tensor.matmul(qps[:, 0:512], warm[:, 0:128],
                                 warm[:, 128:640], start=True, stop=True)
            pe.wait_ge(dma0_sem, 16)
            pe.wait_ge(strt_sem, 16)
            q12(0)
            q12(1)
            pe.wait_ge(dma2_sem, 16)
            pe.wait_ge(dma3_sem, 16)
            q34(0).then_inc(pe_sem, 1)
            q34(1).then_inc(pe_sem, 1)
            for c in range(2, len(chunks)):
                if c == 2:
                    pe.wait_ge(dma1_sem, 16)
                if c >= 4:
                    pe.wait_ge(dve_sem, opidx[c - 4] + 1)
                q12(c)
                q34(c).then_inc(pe_sem, 1)

        @block.scalar
        def _(act):
            act.dma_start(out=feat[:, L:L + 128],
                          in_=d_feat[:, L:L + 128]).then_inc(strt_sem, 16)
            act.dma_start(out=feat[:, 2 * L:3 * L],
                          in_=d_feat[:, 2 * L:3 * L]).then_inc(dma2_sem, 16)
            nc.scalar.activation(out=sring[:, 0:8], in_=sring[:, 0:8],
                                 func=ACT.Sigmoid,
                                 scale=0.0).then_inc(act_sem, 1)
            for oi, (c0, k, pos0, W) in enumerate(ops):
                act.wait_ge(pe12_sem, c0 + k)
                o12 = 512 * (c0 % 4)
                nc.scalar.activation(out=sring[:, pos0:pos0 + W],
                                     in_=qps[:, o12:o12 + W],
                                     func=ACT.Sigmoid,
                                     scale=-0.01).then_inc(act_sem, 1)
            act.wait_ge(dve_sem, len(ops))
            act.dma_start(out=d_out[:, NSPLIT:NG],
                          in_=cols[:, NSPLIT:NG]).then_inc(dmao_sem, 16)

        @block.vector
        def _(dve):
            for oi, (c0, k, pos0, W) in enumerate(ops):
                dve.wait_ge(act_sem, oi + 2)
                dve.wait_ge(pe_sem, c0 + k)
                o34 = 2048 + 512 * (c0 % 4)
                nc.vector.scalar_tensor_tensor(
                    out=sring[:, pos0:pos0 + W],
                    in0=qps[:, o34:o34 + W],
                    scalar=0.0, op0=ALU.is_lt,
                    in1=sring[:, pos0:pos0 + W], op1=ALU.mult,
                    accum_out=cols[:, oi:oi + 1]).then_inc(dve_sem, 1)

    _split_multi_waits(nc)
    return nc


def _sig(x):
    with np.errstate(over="ignore"):
        return 1.0 / (1.0 + np.exp(np.clip(x, -500.0, 500.0)))


def kernel(point_pred, orient_pred, class_pred, target_seq, padding_mask):
    pp = np.ascontiguousarray(np.asarray(point_pred, dtype=np.float32))
    op = np.ascontiguousarray(np.asarray(orient_pred, dtype=np.float32))
    cp = np.ascontiguousarray(np.asarray(class_pred, dtype=np.float32))
    ts = np.ascontiguousarray(np.asarray(target_seq, dtype=np.float32))
    pm = np.ascontiguousarray(np.asarray(padding_mask)).astype(bool)

    tc_cls, valid, nn, per_core, n_segs = _host_prep(pp, ts, pm)

    nsmax = max(max(n_segs), 1)
    L = max(128, -(-nsmax // 128) * 128)
    L = min(L, -(-M // 128) * 128)
    Jmax = min(-(-nsmax // 8) * 8, L)

    key = (L, Jmax)
    if key not in _CACHE:
        _CACHE[key] = _build_program(L, Jmax)
    nc = _CACHE[key]
    chunks, ops, _opidx, _total = _schedule(L, Jmax)

    import ml_dtypes
    bfdt = ml_dtypes.bfloat16
    in_maps = []
    for b in range(B):
        pc = per_core[b]
        featpk = np.zeros((KQ12, 4 * L), bfdt)
        w = min(M, L)
        featpk[:KQ12, 0 * L:0 * L + w] = pc["B12"][:, :w]
        featpk[:KQ12, 1 * L:1 * L + w] = pc["A12"][:, :w]
        featpk[:KQ34, 2 * L:2 * L + w] = pc["B34"][:, :w]
        featpk[:KQ34, 3 * L:3 * L + w] = pc["A34"][:, :w]
        if L > M:
            big_bf = bfdt(BIG)
            featpk[24, 1 * L + M:2 * L] = big_bf
            featpk[25, 1 * L + M:2 * L] = bfdt(1.0)
            featpk[24, 0 * L + M:1 * L] = bfdt(1.0)
            featpk[25, 0 * L + M:1 * L] = big_bf
        in_maps.append({"feat": np.ascontiguousarray(featpk)})

    from concourse.bass_utils import run_bass_kernel_spmd
    global LAST_RESULTS
    kw = dict(TRACE_KWARGS) if TRACE_KWARGS else {}
    res = run_bass_kernel_spmd(nc, in_maps, core_ids=list(range(NCORES)), **kw)
    LAST_RESULTS = res
    parts = [r["partials"] for r in res.results]

    NG = len(ops)
    f32 = np.float32

    isect_sum = np.float64(0.0)
    cnt_total = 0
    nb = L // 128
    for b in range(B):
        pc = per_core[b]
        n, n_seg = pc["n"], pc["n_seg"]
        raw = np.float64(parts[b][:, :NG].astype(np.float64).sum())
        A12 = pc["A12"].astype(np.float32)
        B12 = pc["B12"].astype(np.float32)
        A34 = pc["A34"].astype(np.float32)
        B34 = pc["B34"].astype(np.float32)
        junk = np.float64(0.0)
        for ib in range(nb):
            i0 = 128 * ib
            jw = min(DIAG_W, Jmax - i0)
            if jw <= 0:
                continue
            ia, ib_ = i0, min(i0 + 128, M)
            ja, jb_ = i0, min(i0 + jw, M)
            q12 = A12[:, ia:ib_].T @ B12[:, ja:jb_]
            q34 = A34[:, ia:ib_].T @ B34[:, ja:jb_]
            di = np.arange(ib_ - ia)[:, None]
            dj = np.arange(jb_ - ja)[None, :]
            msk = dj < di + 2
            s = _sig(0.01 * q12.astype(np.float64)) * (q34 < 0.0)
            junk += np.where(msk, s, 0.0).sum()
        wrap = np.float64(0.0)
        if n >= 4:
            jw_ = n_seg - 1
            q12w = np.float64(A12[:, 0] @ B12[:, jw_])
            q34w = np.float64(A34[:, 0] @ B34[:, jw_])
            wrap = _sig(0.01 * q12w) * float(q34w < 0.0)
            cnt_total += (n_seg - 1) * (n_seg - 2) // 2 - 1
            isect_sum += raw - junk - wrap

    if cnt_total > 0:
        isect_loss = f32(isect_sum / cnt_total)
    else:
        isect_loss = f32(0.0)

    valid_f = valid.astype(np.float64)
    nn_f = nn.astype(np.float64)
    vden = max(valid_f.sum(), 1.0)
    nden = max(nn_f.sum(), 1.0)

    x = cp.astype(np.float64)
    xmax = x.max(axis=-1, keepdims=True)
    lse = np.log(np.exp(x - xmax).sum(axis=-1)) + xmax[..., 0]
    sel = np.take_along_axis(x, tc_cls[..., None], axis=-1)[..., 0]
    cls_loss = f32(((lse - sel) * valid_f).sum() / vden)

    d = (pp.astype(np.float64) - ts[:, :, :2].astype(np.float64)) / RETINA
    ad = np.abs(d)
    sl1 = np.where(ad < 1.0, 0.5 * d * d, ad - 0.5).mean(axis=-1)
    pt_loss = f32((sl1 * nn_f).sum() / nden)

    cos = (op.astype(np.float64) * ts[:, :, 2:4].astype(np.float64)).sum(-1)
    orient_loss = f32(((1.0 - cos) * nn_f).sum() / nden)

    total = f32(pt_loss + f32(0.5) * orient_loss + cls_loss
                + f32(0.1) * isect_loss)
    return (total, pt_loss, orient_loss, cls_loss, isect_loss)

